# revision 21
# baseline (speedup 1.0000x reference)
"""Trainium2 Bass kernel for nn_AdaptiveCouplingLayer (Kuramoto coupling layer).

Self-contained: takes FULL inputs, shards over 8 NeuronCores internally
(data-parallel over batch x sequence-half), returns FULL outputs.

Sharding: core c handles batch b = c//2, tokens s in [4096*(c%2), 4096*(c%2)+4096).
Host prep transposes hidden to [H, S_local] per core so the contraction dim (H)
lands on SBUF partitions; w_out is fed in natural [H, O] layout.

Engine plan (strict per-engine FIFO ordering is the design constraint):
  PE:     pf matmuls, main matmuls, K3 epilogue matmuls (bulk stream only)
  ACT:    gctx row-sum accumulators, pf drains, tanh/gelu/sigmoid/sin/sqrt
  DVE:    PSUM->SBUF staging casts, K3 adds
  GPSIMD: collective, partner copies, K-MLP, Kuramoto elementwise,
          cross-partition reduces/broadcasts
Token mapping within a core: t = 32*p + j on the [128, 32] Kuramoto grid,
identical to natural order in the cs3 epilogue rows.
"""
import math
import os

import numpy as np

MIN_K, MAX_K, NUM_STEPS, DT = 0.1, 3.0, 5, 0.01
PI = math.pi

B, S, H = 4, 8192, 1024
N_CORES = 8
TL = S * B // N_CORES  # 4096 tokens per core
NJ = TL // 128         # 32 t-tiles per core
NK = H // 128          # 8 h-tiles (contraction)
INV_S = 1.0 / S

_CACHE = {}

LAST_RESULT = None


def _build(scalars):
    from concourse import bass, bacc, mybir, tile
    from concourse import bass_isa

    F32 = mybir.dt.float32
    F32R = mybir.dt.float32r
    AF = mybir.ActivationFunctionType
    ALU = mybir.AluOpType

    b_phase, b_freq, b_c2 = scalars

    nc = bacc.Bacc("TRN2", target_bir_lowering=False, debug=False,
                   num_devices=N_CORES)

    hidT_ext = nc.declare_dram_parameter("hidT", [H, TL], F32R, isOutput=False)
    w_ext = nc.declare_dram_parameter("w_main", [H, H], F32R, isOutput=False)
    wpib_ext = nc.declare_dram_parameter("w_pi_b", [4, H], F32R, isOutput=False)
    wpf_ext = nc.declare_dram_parameter("wpf", [H, 2], F32R, isOutput=False)
    w1_ext = nc.declare_dram_parameter("w_c1", [128, 8 * 256], F32, isOutput=False)
    w2_ext = nc.declare_dram_parameter("w_c2r", [1, 256], F32, isOutput=False)
    b1_ext = nc.declare_dram_parameter("b_c1r", [1, 256], F32, isOutput=False)
    onesr_ext = nc.declare_dram_parameter("ones_row", [1, TL], F32R, isOutput=False)

    out_ext = nc.declare_dram_parameter("out", [TL, H], F32, isOutput=True)
    r_ext = nc.declare_dram_parameter("r_out", [1, 1], F32, isOutput=True)
    k_ext = nc.declare_dram_parameter("K_out", [1, 1], F32, isOutput=True)

    agin = nc.dram_tensor("agin", [128, 72], F32)
    agout = nc.dram_tensor("agout", [N_CORES * 128, 72], F32, addr_space="Shared")

    with tile.TileContext(nc) as tc:
        with (
            tc.tile_pool(name="res", bufs=1) as res,
            tc.tile_pool(name="psm", bufs=8, space="PSUM") as psm,
        ):
            hidT = res.tile([128, NK * TL], F32R, tag="hidT")
            w_sb = res.tile([128, NK * H], F32R, tag="w")
            wpf_sb = res.tile([128, 2 * NK], F32R, tag="wpf")
            w1_sb = res.tile([128, 8 * 256], F32, tag="w1")
            w2_sb = res.tile([1, 256], F32, tag="w2")
            b1_sb = res.tile([1, 256], F32, tag="b1")
            wcs3_sb = res.tile([3, H], F32R, tag="wcs3")
            wr_sb = res.tile([1, H], F32R, tag="wr")
            b0_sb = res.tile([1, H], F32R, tag="b0")
            wrb_sb = res.tile([1, H], F32R, tag="wrb")
            cs3_sb = res.tile([3, TL], F32R, tag="cs3")
            ph_sb = res.tile([128, 64], F32, tag="ph")
            dtf_sb = res.tile([128, 64], F32, tag="dtf")
            csh_sb = res.tile([128, 64], F32, tag="csh")
            cs_sb = res.tile([128, 128], F32R, tag="cs")
            u1_sb = res.tile([128, 64], F32, tag="u1")
            u2_sb = res.tile([128, 64], F32, tag="u2")
            wm_sb = res.tile([128, 64], F32, tag="wm")
            rsum_sb = res.tile([128, 2], F32, tag="rsum")
            sums_sb = res.tile([128, 2], F32, tag="sums")
            gct_sb = res.tile([128, NK], F32, tag="gct")
            gtmp_sb = res.tile([128, 64], F32, tag="gtmp")
            ptn_sb = res.tile([128, 72], F32, tag="ptn")
            hmid_sb = res.tile([128, 256], F32, tag="hmid")
            hmr_sb = res.tile([1, 256], F32, tag="hmr")
            scr2_sb = res.tile([1, 256], F32, tag="scr2")
            small_sb = res.tile([128, 8], F32, tag="small")
            hpi_col = res.tile([128, 1], F32, tag="hpi")
            mpi_col = res.tile([128, 1], F32, tag="mpi")
            ppi_col = res.tile([128, 1], F32, tag="ppi")
            # small cols: 0=r_col, 1=ndtkn_col; row0: 2=sig, 3=K, 4=ndtkn, 5/6 scratch

            nc.vector.memset(hpi_col[:], PI / 2)
            nc.vector.memset(mpi_col[:], -PI)
            nc.vector.memset(ppi_col[:], PI)

            # ---- constant DMAs ----
            nc.sync.dma_start(
                wpf_sb[:],
                wpf_ext[:].rearrange("(k p) c -> p k c", k=NK, p=128))
            nc.sync.dma_start(w1_sb[:], w1_ext[:])
            nc.sync.dma_start(w2_sb[:], w2_ext[:])
            nc.sync.dma_start(b1_sb[:], b1_ext[:])
            nc.sync.dma_start(wr_sb[:], wpib_ext[2:3, :])
            nc.sync.dma_start(b0_sb[:], wpib_ext[3:4, :])
            nc.sync.dma_start(wcs3_sb[0:2, :], wpib_ext[0:2, :])
            nc.sync.dma_start(cs3_sb[2:3, :], onesr_ext[:])

            hid4 = hidT[:].rearrange("p (k j t) -> p k j t", k=NK, j=NJ, t=128)

            def staged(j, half):
                lo = 4 * half
                return hid4[:, lo:lo + 4, j:j + 1, :]

            def main_group(j):
                ps_halves = []
                for half in range(2):
                    o0 = 512 * half
                    mm_ps = psm.tile([128, 512], F32, tag="mm")
                    for k in range(NK):
                        nc.tensor.matmul(
                            mm_ps[:],
                            hidT[:, k * TL + 128 * j:k * TL + 128 * j + 128],
                            w_sb[:, k * H + o0:k * H + o0 + 512],
                            start=(k == 0), stop=(k == NK - 1),
                        )
                    ps_halves.append(mm_ps)
                for half in range(2):
                    with nc.allow_low_precision(reason="f32r staging"):
                        nc.vector.tensor_copy(staged(j, half), ps_halves[half][:])

            # ---- streaming front: per 512-token block b ----
            for b in range(8):
                for k in range(NK):
                    nc.sync.dma_start(
                        hidT[:, k * TL + 512 * b:k * TL + 512 * b + 512],
                        hidT_ext[128 * k:128 * k + 128, 512 * b:512 * b + 512],
                    )
                if b == 0:
                    for k in range(NK):
                        nc.sync.dma_start(
                            w_sb[:, k * H:(k + 1) * H],
                            w_ext[128 * k:128 * k + 128, :],
                        )
                # gctx partial row-sums: split ACT / DVE
                for k in range(NK):
                    if k < 4:
                        with nc.allow_low_precision(reason="in-place f32r pass"):
                            nc.scalar.activation(
                                hidT[:, k * TL + 512 * b:k * TL + 512 * b + 512],
                                hidT[:, k * TL + 512 * b:k * TL + 512 * b + 512].bitcast(F32),
                                AF.Copy, accum_out=gtmp_sb[:, 8 * k + b:8 * k + b + 1])
                    else:
                        nc.vector.tensor_reduce(
                            out=gtmp_sb[:, 8 * k + b:8 * k + b + 1],
                            in_=hidT[:, k * TL + 512 * b:k * TL + 512 * b + 512].bitcast(F32),
                            axis=mybir.AxisListType.X, op=ALU.add)
                # pf matmuls for this block
                pf_ps = psm.tile([2, 512], F32, tag="mm")
                for k in range(NK):
                    nc.tensor.matmul(
                        pf_ps[:],
                        wpf_sb[:, 2 * k:2 * k + 2],
                        hidT[:, k * TL + 512 * b:k * TL + 512 * b + 512],
                        start=(k == 0), stop=(k == NK - 1),
                    )
                with nc.allow_low_precision(reason="f32r pf rows"):
                    nc.scalar.activation(
                        cs3_sb[0:2, 512 * b:512 * b + 512], pf_ps[:], AF.Copy)
                # one main tile to keep PE paced with the DMA stream
                main_group(4 * b)

            # ---- gctx finish (ACT accum over gtmp columns) ----
            for k in range(NK):
                nc.scalar.activation(
                    gtmp_sb[:, 8 * k:8 * k + 8], gtmp_sb[:, 8 * k:8 * k + 8],
                    AF.Copy, accum_out=gct_sb[:, k:k + 1])

            # ---- ph/dtf shuffles + scalarizations ----
            nc.sync.dma_start(
                ph_sb[:, 0:32],
                cs3_sb[0:1, :].bitcast(F32).rearrange("a (p j) -> a p j", p=128, j=32))
            nc.sync.dma_start(
                dtf_sb[:, 0:32],
                cs3_sb[1:2, :].bitcast(F32).rearrange("a (p j) -> a p j", p=128, j=32))
            nc.scalar.activation(ph_sb[:, 0:32], ph_sb[:, 0:32], AF.Tanh,
                                 bias=float(b_phase), scale=1.0)
            nc.vector.tensor_single_scalar(
                out=ph_sb[:, 0:32], in_=ph_sb[:, 0:32], scalar=PI, op=ALU.mult)
            nc.vector.tensor_scalar(
                out=dtf_sb[:, 0:32], in0=dtf_sb[:, 0:32],
                scalar1=DT, scalar2=DT * float(b_freq),
                op0=ALU.mult, op1=ALU.add)

            # ---- main section A (16 tiles) ----
            for b in range(8):
                for j in (4 * b + 1, 4 * b + 2):
                    main_group(j)

            # ---- AllGather (trigger ASAP) ----
            nc.sync.dma_start(agin[:, 0:32], ph_sb[:, 0:32])
            nc.sync.dma_start(agin[:, 32:64], dtf_sb[:, 0:32])
            nc.sync.dma_start(agin[:, 64:72], gct_sb[:])
            nc.gpsimd.collective_compute(
                "AllGather", ALU.bypass,
                ins=[agin[:].opt()], outs=[agout[:].opt()],
                replica_groups=[list(range(N_CORES))],
            )
            pid = nc.sync.partition_id()
            for c in range(N_CORES):
                t = c ^ 1
                is_partner = (pid >= t) & (pid <= t)
                nc.sync.dma_start(ptn_sb[:], agout[128 * c:128 * c + 128, :],
                                  cond=is_partner)
            nc.vector.tensor_copy(ph_sb[:, 32:64], ptn_sb[:, 0:32])
            nc.vector.tensor_copy(dtf_sb[:, 32:64], ptn_sb[:, 32:64])
            nc.vector.tensor_tensor(out=gct_sb[:], in0=gct_sb[:],
                                    in1=ptn_sb[:, 64:72], op=ALU.add)

            # ---- K MLP (DVE + ACT + gpsimd partition reduce; no PE) ----
            for k in range(NK):
                if k == 0:
                    nc.vector.tensor_single_scalar(
                        out=hmid_sb[:], in_=w1_sb[:, 256 * k:256 * k + 256],
                        scalar=gct_sb[:, k:k + 1], op=ALU.mult)
                else:
                    nc.vector.scalar_tensor_tensor(
                        out=hmid_sb[:], in0=w1_sb[:, 256 * k:256 * k + 256],
                        scalar=gct_sb[:, k:k + 1], in1=hmid_sb[:],
                        op0=ALU.mult, op1=ALU.add)
            nc.gpsimd.partition_all_reduce(
                hmid_sb[:], hmid_sb[:], channels=128,
                reduce_op=bass_isa.ReduceOp.add)
            # x = hmid/S + b_c1 ; hg = gelu(x) (partition 0 only)
            nc.gpsimd.tensor_single_scalar(
                out=hmr_sb[:], in_=hmid_sb[0:1, :], scalar=INV_S, op=ALU.mult)
            nc.gpsimd.tensor_tensor(out=hmr_sb[:], in0=hmr_sb[:],
                                    in1=b1_sb[:], op=ALU.add)
            nc.scalar.activation(hmr_sb[:], hmr_sb[:], AF.Gelu)
            # K_raw = sum(hg * w2)
            nc.gpsimd.tensor_tensor(out=scr2_sb[:], in0=hmr_sb[:],
                                    in1=w2_sb[:], op=ALU.mult)
            nc.scalar.activation(scr2_sb[:], scr2_sb[:], AF.Copy,
                                 accum_out=small_sb[0:1, 5:6])
            nc.scalar.activation(small_sb[0:1, 2:3], small_sb[0:1, 5:6],
                                 AF.Sigmoid, bias=float(b_c2), scale=1.0)
            nc.vector.tensor_scalar(
                out=small_sb[0:1, 3:4], in0=small_sb[0:1, 2:3],
                scalar1=MAX_K - MIN_K, scalar2=MIN_K, op0=ALU.mult, op1=ALU.add)
            nc.vector.tensor_scalar(
                out=small_sb[0:1, 4:5], in0=small_sb[0:1, 3:4],
                scalar1=-DT * INV_S, scalar2=None, op0=ALU.mult)
            nc.sync.dma_start(k_ext[:], small_sb[0:1, 3:4])
            nc.gpsimd.partition_broadcast(small_sb[:, 1:2], small_sb[0:1, 4:5])

            # ---- Kuramoto steps (DVE elementwise + ACT sin + gpsimd reduce) ----
            for step in range(NUM_STEPS + 1):
                last = step == NUM_STEPS
                nc.vector.add_range_wrap(csh_sb[:], ph_sb[:], PI / 2, PI, 2 * PI)
                with nc.allow_low_precision(reason="f32r cs for epilogue"):
                    nc.scalar.activation(cs_sb[:, 0:64], csh_sb[:], AF.Sin,
                                         accum_out=rsum_sb[:, 0:1])
                    nc.scalar.activation(cs_sb[:, 64:128], ph_sb[:], AF.Sin,
                                         accum_out=rsum_sb[:, 1:2])
                nc.gpsimd.partition_all_reduce(
                    sums_sb[:], rsum_sb[:], channels=128,
                    reduce_op=bass_isa.ReduceOp.add)
                if not last:
                    nc.vector.tensor_single_scalar(
                        out=u1_sb[:], in_=cs_sb[:, 64:128].bitcast(F32),
                        scalar=sums_sb[:, 0:1], op=ALU.mult)
                    nc.vector.scalar_tensor_tensor(
                        out=u2_sb[:], in0=cs_sb[:, 0:64].bitcast(F32),
                        scalar=sums_sb[:, 1:2], in1=u1_sb[:],
                        op0=ALU.mult, op1=ALU.subtract)
                    nc.vector.tensor_tensor(out=ph_sb[:], in0=ph_sb[:],
                                            in1=dtf_sb[:], op=ALU.add)
                    nc.vector.scalar_tensor_tensor(
                        out=ph_sb[:], in0=u2_sb[:], scalar=small_sb[:, 1:2],
                        in1=ph_sb[:], op0=ALU.mult, op1=ALU.add)
                    nc.vector.add_range_wrap(ph_sb[:], ph_sb[:], 0.0, PI, 2 * PI)
                else:
                    nc.vector.tensor_single_scalar(
                        out=small_sb[:, 5:6], in_=sums_sb[:, 1:2],
                        scalar=sums_sb[:, 1:2], op=ALU.mult)
                    nc.vector.scalar_tensor_tensor(
                        out=small_sb[:, 6:7], in0=sums_sb[:, 0:1],
                        scalar=sums_sb[:, 0:1], in1=small_sb[:, 5:6],
                        op0=ALU.mult, op1=ALU.add)
                    nc.scalar.activation(small_sb[:, 0:1], small_sb[:, 6:7],
                                         AF.Sqrt, bias=0.0, scale=INV_S * INV_S)
                    nc.sync.dma_start(r_ext[:], small_sb[0:1, 0:1])

            # ---- cs3 rows + wrb row ----
            nc.sync.dma_start(
                cs3_sb[0:1, :].rearrange("a (p j) -> a p j", p=128, j=32),
                cs_sb[:, 0:32])
            nc.sync.dma_start(
                cs3_sb[1:2, :].rearrange("a (p j) -> a p j", p=128, j=32),
                cs_sb[:, 64:96])
            with nc.allow_low_precision(reason="f32r epilogue row"):
                nc.vector.scalar_tensor_tensor(
                    out=wrb_sb[0:1, :], in0=wr_sb[0:1, :].bitcast(F32),
                    scalar=small_sb[0:1, 0:1], in1=b0_sb[0:1, :].bitcast(F32),
                    op0=ALU.mult, op1=ALU.add)
            nc.sync.dma_start(wcs3_sb[2:3, :], wrb_sb[0:1, :])


            # ---- main section B (8 tiles) ----
            for b in range(8):
                main_group(4 * b + 3)

            # ---- phase epilogue: K3 matmul + DVE add + out DMA ----
            for j in range(NJ):
                for half in range(2):
                    o0 = 512 * half
                    k3_ps = psm.tile([128, 512], F32, tag="mm")
                    nc.tensor.matmul(
                        k3_ps[:],
                        cs3_sb[:, 128 * j:128 * j + 128],
                        wcs3_sb[:, o0:o0 + 512],
                        start=True, stop=True,
                    )
                    with nc.allow_low_precision(reason="f32r staging add"):
                        nc.vector.tensor_tensor(
                            out=staged(j, half), in0=staged(j, half).bitcast(F32),
                            in1=k3_ps[:], op=ALU.add)
                    nc.sync.dma_start(
                        out_ext[128 * j:128 * j + 128, 512 * half:512 * half + 512],
                        hid4[:, 4 * half:4 * half + 4, j:j + 1, :].bitcast(F32),
                    )

    nc.compile()
    return nc


def _get_nc(scalars):
    key = tuple(float(x) for x in scalars)
    if key not in _CACHE:
        _CACHE[key] = _build(key)
    return _CACHE[key]


def kernel(hidden_states, w_c1, b_c1, w_c2, b_c2, w_phase, b_phase,
           w_freq, b_freq, w_out, b_out):
    global LAST_RESULT
    from concourse.bass_utils import run_bass_kernel_spmd

    hidden_states = np.asarray(hidden_states, dtype=np.float32)
    w_c1 = np.asarray(w_c1, dtype=np.float32)
    b_c1 = np.asarray(b_c1, dtype=np.float32)
    w_c2 = np.asarray(w_c2, dtype=np.float32)
    b_c2 = np.asarray(b_c2, dtype=np.float32)
    w_phase = np.asarray(w_phase, dtype=np.float32)
    b_phase = np.asarray(b_phase, dtype=np.float32)
    w_freq = np.asarray(w_freq, dtype=np.float32)
    b_freq = np.asarray(b_freq, dtype=np.float32)
    w_out = np.asarray(w_out, dtype=np.float32)
    b_out = np.asarray(b_out, dtype=np.float32)

    nc = _get_nc((float(b_phase[0]), float(b_freq[0]), float(b_c2[0])))

    w_main = np.ascontiguousarray(w_out[0:H, :])
    w_pi_b = np.ascontiguousarray(
        np.concatenate([w_out[H:H + 3, :], b_out[None, :]], axis=0))
    wpf = np.ascontiguousarray(np.concatenate([w_phase, w_freq], axis=1))
    # w1[p, k*256 + m] = w_c1[128k + p, m]
    w1 = np.ascontiguousarray(
        w_c1.reshape(NK, 128, 256).transpose(1, 0, 2).reshape(128, NK * 256))
    w2r = np.ascontiguousarray(w_c2.reshape(1, 256))
    b1r = np.ascontiguousarray(b_c1.reshape(1, 256))
    ones_row = np.ones((1, TL), dtype=np.float32)

    in_maps = []
    for c in range(N_CORES):
        b = c // 2
        s0 = TL * (c % 2)
        hidT = np.ascontiguousarray(hidden_states[b, s0:s0 + TL, :].T)
        in_maps.append({
            "hidT": hidT, "w_main": w_main, "w_pi_b": w_pi_b, "wpf": wpf,
            "w_c1": w1, "w_c2r": w2r, "b_c1r": b1r, "ones_row": ones_row,
        })

    res = run_bass_kernel_spmd(nc, in_maps, list(range(N_CORES)),
                               trace=os.environ.get("BASS_TRACE", "0") == "1")
    LAST_RESULT = res

    output = np.empty((B, S, H), dtype=np.float32)
    r = np.empty((B,), dtype=np.float32)
    K = np.empty((B,), dtype=np.float32)
    for c in range(N_CORES):
        b = c // 2
        s0 = TL * (c % 2)
        output[b, s0:s0 + TL, :] = res.results[c]["out"]
        if c % 2 == 0:
            r[b] = res.results[c]["r_out"][0, 0]
            K[b] = res.results[c]["K_out"][0, 0]
    return output, r, K


# revision 23
# speedup vs baseline: 1.0940x; 1.0940x over previous
"""Trainium2 Bass kernel for nn_AdaptiveCouplingLayer (Kuramoto coupling layer).

Self-contained: takes FULL inputs, shards over 8 NeuronCores internally
(data-parallel over batch x sequence-half), returns FULL outputs.

Sharding: core c handles batch b = c//2, tokens s in [4096*(c%2), 4096*(c%2)+4096).
Host prep transposes hidden to [H, S_local] per core so the contraction dim (H)
lands on SBUF partitions; w_out is fed in natural [H, O] layout.

Engine plan (strict per-engine FIFO ordering is the design constraint):
  PE:     pf matmuls, main matmuls, K3 epilogue matmuls (bulk stream only)
  ACT:    gctx row-sum accumulators, pf drains, tanh/gelu/sigmoid/sin/sqrt
  DVE:    PSUM->SBUF staging casts, K3 adds
  GPSIMD: collective, partner copies, K-MLP, Kuramoto elementwise,
          cross-partition reduces/broadcasts
Token mapping within a core: t = 32*p + j on the [128, 32] Kuramoto grid,
identical to natural order in the cs3 epilogue rows.
"""
import math
import os

import numpy as np

MIN_K, MAX_K, NUM_STEPS, DT = 0.1, 3.0, 5, 0.01
PI = math.pi

B, S, H = 4, 8192, 1024
N_CORES = 8
TL = S * B // N_CORES  # 4096 tokens per core
NJ = TL // 128         # 32 t-tiles per core
NK = H // 128          # 8 h-tiles (contraction)
INV_S = 1.0 / S

_CACHE = {}

LAST_RESULT = None


def _build(scalars):
    from concourse import bass, bacc, mybir, tile
    from concourse import bass_isa

    F32 = mybir.dt.float32
    F32R = mybir.dt.float32r
    AF = mybir.ActivationFunctionType
    ALU = mybir.AluOpType

    b_phase, b_freq, b_c2 = scalars

    nc = bacc.Bacc("TRN2", target_bir_lowering=False, debug=False,
                   num_devices=N_CORES)

    hidT_ext = nc.declare_dram_parameter("hidT", [H, TL], F32R, isOutput=False)
    w_ext = nc.declare_dram_parameter("w_main", [H, H], F32R, isOutput=False)
    wpib_ext = nc.declare_dram_parameter("w_pi_b", [4, H], F32R, isOutput=False)
    wpf_ext = nc.declare_dram_parameter("wpf", [H, 2], F32R, isOutput=False)
    w1_ext = nc.declare_dram_parameter("w_c1", [128, 8 * 256], F32, isOutput=False)
    w2_ext = nc.declare_dram_parameter("w_c2r", [1, 256], F32, isOutput=False)
    b1_ext = nc.declare_dram_parameter("b_c1r", [1, 256], F32, isOutput=False)
    onesr_ext = nc.declare_dram_parameter("ones_row", [1, TL], F32R, isOutput=False)

    out_ext = nc.declare_dram_parameter("out", [TL, H], F32, isOutput=True)
    r_ext = nc.declare_dram_parameter("r_out", [1, 1], F32, isOutput=True)
    k_ext = nc.declare_dram_parameter("K_out", [1, 1], F32, isOutput=True)

    agin = nc.dram_tensor("agin", [128, 64], F32)
    agout = nc.dram_tensor("agout", [N_CORES * 128, 64], F32, addr_space="Shared")
    aginB = nc.dram_tensor("aginB", [128, 8], F32)
    agoutB = nc.dram_tensor("agoutB", [N_CORES * 128, 8], F32, addr_space="Shared")

    with tile.TileContext(nc) as tc:
        with (
            tc.tile_pool(name="res", bufs=1) as res,
            tc.tile_pool(name="psm", bufs=8, space="PSUM") as psm,
        ):
            hidT = res.tile([128, NK * TL], F32R, tag="hidT")
            w_sb = res.tile([128, NK * H], F32R, tag="w")
            wpf_sb = res.tile([128, 2 * NK], F32R, tag="wpf")
            w1_sb = res.tile([128, 8 * 256], F32, tag="w1")
            w2_sb = res.tile([1, 256], F32, tag="w2")
            b1_sb = res.tile([1, 256], F32, tag="b1")
            wcs3_sb = res.tile([3, H], F32R, tag="wcs3")
            wr_sb = res.tile([1, H], F32R, tag="wr")
            b0_sb = res.tile([1, H], F32R, tag="b0")
            wrb_sb = res.tile([1, H], F32R, tag="wrb")
            cs3_sb = res.tile([3, TL], F32R, tag="cs3")
            ph_sb = res.tile([128, 64], F32, tag="ph")
            dtf_sb = res.tile([128, 64], F32, tag="dtf")
            csh_sb = res.tile([128, 64], F32, tag="csh")
            cs_sb = res.tile([128, 128], F32R, tag="cs")
            u1_sb = res.tile([128, 64], F32, tag="u1")
            u2_sb = res.tile([128, 64], F32, tag="u2")
            wm_sb = res.tile([128, 64], F32, tag="wm")
            rsum_sb = res.tile([128, 2], F32, tag="rsum")
            sums_sb = res.tile([128, 2], F32, tag="sums")
            gct_sb = res.tile([128, NK], F32, tag="gct")
            gtmp_sb = res.tile([128, 64], F32, tag="gtmp")
            ptn_sb = res.tile([128, 72], F32, tag="ptn")
            hmid_sb = res.tile([128, 256], F32, tag="hmid")
            hmr_sb = res.tile([1, 256], F32, tag="hmr")
            scr2_sb = res.tile([1, 256], F32, tag="scr2")
            small_sb = res.tile([128, 8], F32, tag="small")
            hpi_col = res.tile([128, 1], F32, tag="hpi")
            mpi_col = res.tile([128, 1], F32, tag="mpi")
            ppi_col = res.tile([128, 1], F32, tag="ppi")
            # small cols: 0=r_col, 1=ndtkn_col; row0: 2=sig, 3=K, 4=ndtkn, 5/6 scratch

            nc.vector.memset(hpi_col[:], PI / 2)
            nc.vector.memset(mpi_col[:], -PI)
            nc.vector.memset(ppi_col[:], PI)
            pid = nc.sync.partition_id()
            # warm the ACT function tables used later
            nc.scalar.activation(small_sb[0:1, 7:8], hpi_col[0:1, :], AF.Tanh)
            nc.scalar.activation(small_sb[0:1, 7:8], hpi_col[0:1, :], AF.Sin)
            nc.scalar.activation(small_sb[0:1, 7:8], hpi_col[0:1, :], AF.Gelu)
            nc.scalar.activation(small_sb[0:1, 7:8], hpi_col[0:1, :], AF.Sigmoid)
            nc.scalar.activation(small_sb[0:1, 7:8], hpi_col[0:1, :], AF.Sqrt)

            # ---- constant DMAs ----
            nc.sync.dma_start(
                wpf_sb[:],
                wpf_ext[:].rearrange("(k p) c -> p k c", k=NK, p=128))
            nc.sync.dma_start(w1_sb[:], w1_ext[:])
            nc.sync.dma_start(w2_sb[:], w2_ext[:])
            nc.sync.dma_start(b1_sb[:], b1_ext[:])
            nc.sync.dma_start(wr_sb[:], wpib_ext[2:3, :])
            nc.sync.dma_start(b0_sb[:], wpib_ext[3:4, :])
            nc.sync.dma_start(wcs3_sb[0:2, :], wpib_ext[0:2, :])
            nc.sync.dma_start(cs3_sb[2:3, :], onesr_ext[:])

            hid4 = hidT[:].rearrange("p (k j t) -> p k j t", k=NK, j=NJ, t=128)

            def staged(j, half):
                lo = 4 * half
                return hid4[:, lo:lo + 4, j:j + 1, :]

            def main_mms(j):
                ps_halves = []
                for half in range(2):
                    o0 = 512 * half
                    mm_ps = psm.tile([128, 512], F32, tag="mm")
                    for k in range(NK):
                        nc.tensor.matmul(
                            mm_ps[:],
                            hidT[:, k * TL + 128 * j:k * TL + 128 * j + 128],
                            w_sb[:, k * H + o0:k * H + o0 + 512],
                            start=(k == 0), stop=(k == NK - 1),
                        )
                    ps_halves.append(mm_ps)
                return ps_halves

            def main_casts(j, ps_halves):
                for half in range(2):
                    with nc.allow_low_precision(reason="f32r staging"):
                        nc.vector.tensor_copy(staged(j, half), ps_halves[half][:])

            def main_group(j):
                main_casts(j, main_mms(j))

            # ---- streaming front: per 512-token block b ----
            blk_ps = {}
            for b in range(8):
                for k in range(NK):
                    nc.sync.dma_start(
                        hidT[:, k * TL + 512 * b:k * TL + 512 * b + 512],
                        hidT_ext[128 * k:128 * k + 128, 512 * b:512 * b + 512],
                    )
                if b == 0:
                    for k in range(NK):
                        nc.sync.dma_start(
                            w_sb[:, k * H:(k + 1) * H],
                            w_ext[128 * k:128 * k + 128, :],
                        )

                def gctx_accums(b=b):
                    for k in range(NK):
                        if k < 4:
                            with nc.allow_low_precision(reason="in-place f32r"):
                                nc.scalar.activation(
                                    hidT[:, k * TL + 512 * b:k * TL + 512 * b + 512],
                                    hidT[:, k * TL + 512 * b:k * TL + 512 * b + 512].bitcast(F32),
                                    AF.Copy,
                                    accum_out=gtmp_sb[:, 8 * k + b:8 * k + b + 1])
                        else:
                            nc.vector.tensor_reduce(
                                out=gtmp_sb[:, 8 * k + b:8 * k + b + 1],
                                in_=hidT[:, k * TL + 512 * b:k * TL + 512 * b + 512].bitcast(F32),
                                axis=mybir.AxisListType.X, op=ALU.add)

                def pf_part(b=b):
                    pf_ps = psm.tile([2, 512], F32, tag="mm")
                    for k in range(NK):
                        nc.tensor.matmul(
                            pf_ps[:],
                            wpf_sb[:, 2 * k:2 * k + 2],
                            hidT[:, k * TL + 512 * b:k * TL + 512 * b + 512],
                            start=(k == 0), stop=(k == NK - 1),
                        )
                    with nc.allow_low_precision(reason="f32r pf rows"):
                        nc.scalar.activation(
                            cs3_sb[0:2, 512 * b:512 * b + 512], pf_ps[:], AF.Copy)

                if b < 7:
                    gctx_accums()
                    pf_part()
                else:
                    # last block: drain pf first so the AllGather can launch
                    pf_part()
                    gctx_accums()
                blk_ps[b] = main_mms(4 * b)
            for b in range(8):
                main_casts(4 * b, blk_ps[b])

            # ---- gctx finish (ACT accum over gtmp columns) ----
            for k in range(NK):
                nc.scalar.activation(
                    gtmp_sb[:, 8 * k:8 * k + 8], gtmp_sb[:, 8 * k:8 * k + 8],
                    AF.Copy, accum_out=gct_sb[:, k:k + 1])

            # ---- ph/dtf shuffles + scalarizations ----
            nc.sync.dma_start(
                ph_sb[:, 0:32],
                cs3_sb[0:1, :].bitcast(F32).rearrange("a (p j) -> a p j", p=128, j=32))
            nc.sync.dma_start(
                dtf_sb[:, 0:32],
                cs3_sb[1:2, :].bitcast(F32).rearrange("a (p j) -> a p j", p=128, j=32))
            nc.scalar.activation(ph_sb[:, 0:32], ph_sb[:, 0:32], AF.Tanh,
                                 bias=float(b_phase), scale=1.0)
            nc.scalar.activation(ph_sb[:, 0:32], ph_sb[:, 0:32], AF.Copy,
                                 bias=0.0, scale=PI)
            nc.scalar.activation(dtf_sb[:, 0:32], dtf_sb[:, 0:32], AF.Copy,
                                 bias=float(DT * b_freq), scale=DT)

            # ---- main section A (16 tiles) ----
            for b in range(8):
                for j in (4 * b + 1, 4 * b + 2):
                    main_group(j)

            # ---- AllGather (trigger ASAP) ----
            nc.sync.dma_start(agin[:, 0:32], ph_sb[:, 0:32])
            nc.sync.dma_start(agin[:, 32:64], dtf_sb[:, 0:32])
            nc.gpsimd.collective_compute(
                "AllGather", ALU.bypass,
                ins=[agin[:].opt()], outs=[agout[:].opt()],
                replica_groups=[list(range(N_CORES))],
            )
            nc.sync.dma_start(aginB[:], gct_sb[:])
            nc.gpsimd.collective_compute(
                "AllGather", ALU.bypass,
                ins=[aginB[:].opt()], outs=[agoutB[:].opt()],
                replica_groups=[list(range(N_CORES))],
            )
            for c in range(N_CORES):
                t = c ^ 1
                is_partner = (pid >= t) & (pid <= t)
                nc.sync.dma_start(ptn_sb[:, 0:64], agout[128 * c:128 * c + 128, :],
                                  cond=is_partner)
            for c in range(N_CORES):
                t = c ^ 1
                is_partner = (pid >= t) & (pid <= t)
                nc.sync.dma_start(ptn_sb[:, 64:72], agoutB[128 * c:128 * c + 128, :],
                                  cond=is_partner)
            nc.vector.tensor_copy(ph_sb[:, 32:64], ptn_sb[:, 0:32])
            nc.vector.tensor_copy(dtf_sb[:, 32:64], ptn_sb[:, 32:64])
            nc.vector.tensor_tensor(out=gct_sb[:], in0=gct_sb[:],
                                    in1=ptn_sb[:, 64:72], op=ALU.add)

            # ---- K MLP (DVE + ACT + gpsimd partition reduce; no PE) ----
            for k in range(NK):
                if k == 0:
                    nc.vector.tensor_single_scalar(
                        out=hmid_sb[:], in_=w1_sb[:, 256 * k:256 * k + 256],
                        scalar=gct_sb[:, k:k + 1], op=ALU.mult)
                else:
                    nc.vector.scalar_tensor_tensor(
                        out=hmid_sb[:], in0=w1_sb[:, 256 * k:256 * k + 256],
                        scalar=gct_sb[:, k:k + 1], in1=hmid_sb[:],
                        op0=ALU.mult, op1=ALU.add)
            nc.gpsimd.partition_all_reduce(
                hmid_sb[:], hmid_sb[:], channels=128,
                reduce_op=bass_isa.ReduceOp.add)
            # x = hmid/S + b_c1 ; hg = gelu(x) (partition 0 only)
            nc.gpsimd.tensor_single_scalar(
                out=hmr_sb[:], in_=hmid_sb[0:1, :], scalar=INV_S, op=ALU.mult)
            nc.gpsimd.tensor_tensor(out=hmr_sb[:], in0=hmr_sb[:],
                                    in1=b1_sb[:], op=ALU.add)
            nc.scalar.activation(hmr_sb[:], hmr_sb[:], AF.Gelu)
            # K_raw = sum(hg * w2)
            nc.gpsimd.tensor_tensor(out=scr2_sb[:], in0=hmr_sb[:],
                                    in1=w2_sb[:], op=ALU.mult)
            nc.scalar.activation(scr2_sb[:], scr2_sb[:], AF.Copy,
                                 accum_out=small_sb[0:1, 5:6])
            nc.scalar.activation(small_sb[0:1, 2:3], small_sb[0:1, 5:6],
                                 AF.Sigmoid, bias=float(b_c2), scale=1.0)
            nc.vector.tensor_scalar(
                out=small_sb[0:1, 3:4], in0=small_sb[0:1, 2:3],
                scalar1=MAX_K - MIN_K, scalar2=MIN_K, op0=ALU.mult, op1=ALU.add)
            nc.vector.tensor_scalar(
                out=small_sb[0:1, 4:5], in0=small_sb[0:1, 3:4],
                scalar1=-DT * INV_S, scalar2=None, op0=ALU.mult)
            nc.sync.dma_start(k_ext[:], small_sb[0:1, 3:4])
            nc.gpsimd.partition_broadcast(small_sb[:, 1:2], small_sb[0:1, 4:5])

            # ---- Kuramoto steps (DVE elementwise + ACT sin + gpsimd reduce) ----
            for step in range(NUM_STEPS + 1):
                last = step == NUM_STEPS
                nc.vector.add_range_wrap(csh_sb[:], ph_sb[:], PI / 2, PI, 2 * PI)
                with nc.allow_low_precision(reason="f32r cs for epilogue"):
                    nc.scalar.activation(cs_sb[:, 0:64], csh_sb[:], AF.Sin,
                                         accum_out=rsum_sb[:, 0:1])
                    nc.scalar.activation(cs_sb[:, 64:128], ph_sb[:], AF.Sin,
                                         accum_out=rsum_sb[:, 1:2])
                nc.gpsimd.partition_all_reduce(
                    sums_sb[:], rsum_sb[:], channels=128,
                    reduce_op=bass_isa.ReduceOp.add)
                if not last:
                    nc.vector.tensor_single_scalar(
                        out=u1_sb[:], in_=cs_sb[:, 64:128].bitcast(F32),
                        scalar=sums_sb[:, 0:1], op=ALU.mult)
                    nc.vector.scalar_tensor_tensor(
                        out=u2_sb[:], in0=cs_sb[:, 0:64].bitcast(F32),
                        scalar=sums_sb[:, 1:2], in1=u1_sb[:],
                        op0=ALU.mult, op1=ALU.subtract)
                    nc.vector.tensor_tensor(out=ph_sb[:], in0=ph_sb[:],
                                            in1=dtf_sb[:], op=ALU.add)
                    nc.vector.scalar_tensor_tensor(
                        out=ph_sb[:], in0=u2_sb[:], scalar=small_sb[:, 1:2],
                        in1=ph_sb[:], op0=ALU.mult, op1=ALU.add)
                    nc.vector.add_range_wrap(ph_sb[:], ph_sb[:], 0.0, PI, 2 * PI)
                else:
                    nc.vector.tensor_single_scalar(
                        out=small_sb[:, 5:6], in_=sums_sb[:, 1:2],
                        scalar=sums_sb[:, 1:2], op=ALU.mult)
                    nc.vector.scalar_tensor_tensor(
                        out=small_sb[:, 6:7], in0=sums_sb[:, 0:1],
                        scalar=sums_sb[:, 0:1], in1=small_sb[:, 5:6],
                        op0=ALU.mult, op1=ALU.add)
                    nc.scalar.activation(small_sb[:, 0:1], small_sb[:, 6:7],
                                         AF.Sqrt, bias=0.0, scale=INV_S * INV_S)
                    nc.sync.dma_start(r_ext[:], small_sb[0:1, 0:1])

            # ---- cs3 rows + wrb row ----
            nc.sync.dma_start(
                cs3_sb[0:1, :].rearrange("a (p j) -> a p j", p=128, j=32),
                cs_sb[:, 0:32])
            nc.sync.dma_start(
                cs3_sb[1:2, :].rearrange("a (p j) -> a p j", p=128, j=32),
                cs_sb[:, 64:96])
            with nc.allow_low_precision(reason="f32r epilogue row"):
                nc.vector.scalar_tensor_tensor(
                    out=wrb_sb[0:1, :], in0=wr_sb[0:1, :].bitcast(F32),
                    scalar=small_sb[0:1, 0:1], in1=b0_sb[0:1, :].bitcast(F32),
                    op0=ALU.mult, op1=ALU.add)
            nc.sync.dma_start(wcs3_sb[2:3, :], wrb_sb[0:1, :])


            # ---- main section B (8 tiles) ----
            for b in range(8):
                main_group(4 * b + 3)

            # ---- phase epilogue: K3 matmul + DVE add + out DMA ----
            for j in range(NJ):
                for half in range(2):
                    o0 = 512 * half
                    k3_ps = psm.tile([128, 512], F32, tag="mm")
                    nc.tensor.matmul(
                        k3_ps[:],
                        cs3_sb[:, 128 * j:128 * j + 128],
                        wcs3_sb[:, o0:o0 + 512],
                        start=True, stop=True,
                    )
                    with nc.allow_low_precision(reason="f32r staging add"):
                        nc.vector.tensor_tensor(
                            out=staged(j, half), in0=staged(j, half).bitcast(F32),
                            in1=k3_ps[:], op=ALU.add)
                    nc.sync.dma_start(
                        out_ext[128 * j:128 * j + 128, 512 * half:512 * half + 512],
                        hid4[:, 4 * half:4 * half + 4, j:j + 1, :].bitcast(F32),
                    )

    nc.compile()
    return nc


def _get_nc(scalars):
    key = tuple(float(x) for x in scalars)
    if key not in _CACHE:
        _CACHE[key] = _build(key)
    return _CACHE[key]


def kernel(hidden_states, w_c1, b_c1, w_c2, b_c2, w_phase, b_phase,
           w_freq, b_freq, w_out, b_out):
    global LAST_RESULT
    from concourse.bass_utils import run_bass_kernel_spmd

    hidden_states = np.asarray(hidden_states, dtype=np.float32)
    w_c1 = np.asarray(w_c1, dtype=np.float32)
    b_c1 = np.asarray(b_c1, dtype=np.float32)
    w_c2 = np.asarray(w_c2, dtype=np.float32)
    b_c2 = np.asarray(b_c2, dtype=np.float32)
    w_phase = np.asarray(w_phase, dtype=np.float32)
    b_phase = np.asarray(b_phase, dtype=np.float32)
    w_freq = np.asarray(w_freq, dtype=np.float32)
    b_freq = np.asarray(b_freq, dtype=np.float32)
    w_out = np.asarray(w_out, dtype=np.float32)
    b_out = np.asarray(b_out, dtype=np.float32)

    nc = _get_nc((float(b_phase[0]), float(b_freq[0]), float(b_c2[0])))

    w_main = np.ascontiguousarray(w_out[0:H, :])
    w_pi_b = np.ascontiguousarray(
        np.concatenate([w_out[H:H + 3, :], b_out[None, :]], axis=0))
    wpf = np.ascontiguousarray(np.concatenate([w_phase, w_freq], axis=1))
    # w1[p, k*256 + m] = w_c1[128k + p, m]
    w1 = np.ascontiguousarray(
        w_c1.reshape(NK, 128, 256).transpose(1, 0, 2).reshape(128, NK * 256))
    w2r = np.ascontiguousarray(w_c2.reshape(1, 256))
    b1r = np.ascontiguousarray(b_c1.reshape(1, 256))
    ones_row = np.ones((1, TL), dtype=np.float32)

    in_maps = []
    for c in range(N_CORES):
        b = c // 2
        s0 = TL * (c % 2)
        hidT = np.ascontiguousarray(hidden_states[b, s0:s0 + TL, :].T)
        in_maps.append({
            "hidT": hidT, "w_main": w_main, "w_pi_b": w_pi_b, "wpf": wpf,
            "w_c1": w1, "w_c2r": w2r, "b_c1r": b1r, "ones_row": ones_row,
        })

    res = run_bass_kernel_spmd(nc, in_maps, list(range(N_CORES)),
                               trace=os.environ.get("BASS_TRACE", "0") == "1")
    LAST_RESULT = res

    output = np.empty((B, S, H), dtype=np.float32)
    r = np.empty((B,), dtype=np.float32)
    K = np.empty((B,), dtype=np.float32)
    for c in range(N_CORES):
        b = c // 2
        s0 = TL * (c % 2)
        output[b, s0:s0 + TL, :] = res.results[c]["out"]
        if c % 2 == 0:
            r[b] = res.results[c]["r_out"][0, 0]
            K[b] = res.results[c]["K_out"][0, 0]
    return output, r, K
